# revision 39
# baseline (speedup 1.0000x reference)
"""Trainium2 Bass kernel for nn_MultiHeadAttention_72765335929540.

Reference semantics (B=8, S=2048, D=512, H=8 identical heads, d_k=d_v=64):
    q = query @ Wq + bq;  k = key @ Wk + bk;  v = key @ Wv + bv   (bug: v from key)
    scores = q k^T / 8 (+ causal mask if training);  att = softmax(scores)
    head = att @ v;  out = tile(head, 8) @ Wo + bo = head @ Wo_eff + bo
where Wo_eff = sum_h Wo[64h:64h+64].  `value` is never read.

Distribution: data-parallel, one batch element per NeuronCore (8 cores).

v4 design: host-transposed bf16 inputs ([128, 4, S] chunk-major), loaded
in column-quarters through HWDGE so the first projection starts ~4us
after launch and k-quarters land just-in-time for their sweep.  All
constants arrive in one packed DMA.  Scores and heads are interleaved
at piece granularity (heads lag scores by 3 pieces) so PE has
independent work at every ACT-exp stall point.  Causal diag mask is a
lower-tri 0/1 multiply on Pool.  Output is stored bf16 (one DMA per
512-row sweep) and upcast on host.

Engine budget: PE matmuls only; ACT exp only; DVE psum evictions
(projection bias-add, v', ht4, out normalize) + reciprocal; Pool diag
masks + memsets.

PSUM (8 banks): sc x4 (proj + scoresT), ha x1 (headT' acc), pl x1
(v' transposes + l column), po x2 (final out psum).
"""
import sys

sys.path.insert(0, "/opt/trn_rl_repo")

import numpy as np
import ml_dtypes

import concourse.bass as bass
import concourse.mybir as mybir
import concourse.tile as tile
from concourse.bass_utils import run_bass_kernel_spmd

BF = mybir.dt.bfloat16
F32 = mybir.dt.float32
S, D, DK = 2048, 512, 64
NB = S // 128          # 16 blocks of 128
H = 8

# packed-constant column layout (bf16 columns)
_C_FRHS = 0            # [65, 512]
_C_MASK = 512          # [128, 128] lower-tri 1/0
_C_ID = 640            # [128, 128] identity
_C_WQ = 768            # [128, 256] wq chunk-major
_C_WKV = 1024          # [128, 512] wkv chunk-major
_C_BIAS = 1536         # [128, 4] = bq (f32 pair), bkv (f32 pair)
_C_TOT = 1540

# ---------------------------------------------------------------------------
# walrus workaround: this build's ISA structs hold few semaphore waits per
# instruction; split the excess onto same-engine NoOps (1 wait each).
_ws_counter = [0]
_CTRL_TYPES = ("InstDrain", "InstNoOp", "InstEventSemaphore", "InstBranch")


def _split_sync_waits(nc, max_waits=1, max_updates=2):
    for f in nc.m.functions:
        for blk in f.blocks:
            insts = blk.instructions
            i = 0
            while i < len(insts):
                inst = insts[i]
                si = inst.sync_info
                if si is None:
                    i += 1
                    continue
                ctrl = type(inst).__name__ in _CTRL_TYPES
                max_w = 1 if ctrl else max_waits
                max_u = 1 if ctrl else max_updates
                waits = list(si.on_wait)
                updates = list(si.on_update)
                if len(waits) <= max_w and len(updates) <= max_u:
                    i += 1
                    continue
                keep_w = waits[-max_w:] if len(waits) > max_w else waits
                extra_w = waits[:-max_w] if len(waits) > max_w else []
                keep_u = updates[:max_u] if len(updates) > max_u else updates
                extra_u = updates[max_u:] if len(updates) > max_u else []
                inst.sync_info = mybir.SyncInfo(on_wait=keep_w, on_update=keep_u)
                pre, post = [], []
                for w in extra_w:
                    _ws_counter[0] += 1
                    nop = mybir.InstNoOp(name=f"WSPLIT-{_ws_counter[0]}", ins=[], outs=[])
                    nop.engine = inst.engine
                    nop.sync_info = mybir.SyncInfo(on_wait=[w], on_update=[])
                    pre.append(nop)
                for u in extra_u:
                    _ws_counter[0] += 1
                    nop = mybir.InstNoOp(name=f"USPLIT-{_ws_counter[0]}", ins=[], outs=[])
                    nop.engine = inst.engine
                    nop.sync_info = mybir.SyncInfo(on_wait=[], on_update=[u])
                    post.append(nop)
                for k, nop in enumerate(pre):
                    insts.insert(i + k, nop)
                for k, nop in enumerate(post):
                    insts.insert(i + len(pre) + 1 + k, nop)
                i += len(pre) + 1 + len(post)


# ---------------------------------------------------------------------------
def _build_nc(masked: bool):
    nc = bass.Bass()
    # quarter-major: [qtr][partition][chunk][512 cols] -> 4KB contiguous rows
    qt_d = nc.declare_dram_parameter("qt", [4, 128, 4, 512], BF, isOutput=False)
    kt_d = nc.declare_dram_parameter("kt", [4, 128, 4, 512], BF, isOutput=False)
    cst_d = nc.declare_dram_parameter("cst", [128, _C_TOT], BF, isOutput=False)
    out_d = nc.declare_dram_parameter("out", [S, D], BF, isOutput=True)
    warm_d = nc.declare_dram_parameter("warm", [128, 1], F32, isOutput=True)

    Exp = mybir.ActivationFunctionType.Exp
    Mult = mybir.AluOpType.mult
    Add = mybir.AluOpType.add

    with tile.TileContext(nc) as tc:
        with (
            tc.tile_pool(name="pers", bufs=1) as pers,
            tc.tile_pool(name="hts", bufs=3) as hts,
            tc.tile_pool(name="osb", bufs=2) as osb,
            tc.tile_pool(name="ps", bufs=2, space="PSUM") as ps,
        ):
            # ---- persistent SBUF ------------------------------------------
            xq = pers.tile([128, 4, S], BF, tag="xq")
            xk = pers.tile([128, 4, S], BF, tag="xk")
            cst_sb = pers.tile([128, _C_TOT], BF, tag="cst")
            frhs_sb = cst_sb[0:DK + 1, _C_FRHS:_C_FRHS + 512]
            ltmask_sb = cst_sb[:, _C_MASK:_C_MASK + 128]
            id_sb = cst_sb[:, _C_ID:_C_ID + 128]
            wq_sb = cst_sb[:, _C_WQ:_C_WQ + 256]
            wkv_sb = cst_sb[:, _C_WKV:_C_WKV + 512]
            bq_sb = cst_sb[0:DK, _C_BIAS:_C_BIAS + 2].bitcast(F32)
            bkv_sb = cst_sb[:, _C_BIAS + 2:_C_BIAS + 4].bitcast(F32)
            qT = pers.tile([DK, S], BF, tag="qT")
            kvT = pers.tile([128, S], BF, tag="kvT")
            # v' per sweep: [128, 4 j-blocks, 64 v + 1 ones + pad]
            vp4 = [pers.tile([128, 4, 66], BF, tag=f"vp{p}", name=f"vp{p}")
                   for p in range(4)]
            Ws = [(S - 128 * J) if masked else S for J in range(NB)]
            pts = [pers.tile([128, Ws[J]], BF, tag=f"pt{J}", name=f"pt_{J}")
                   for J in range(NB)]
            wu = pers.tile([128, 512], BF, tag="wu")
            wu2 = pers.tile([128, 1], F32, tag="wu2")

            # ---- loads: early quarters on sync (HWDGE), late on Pool ------
            def load_qtr(x_sb, src_d, qtr, eng):
                sl = slice(qtr * 512, (qtr + 1) * 512)
                eng.dma_start(x_sb[:, :, sl], src_d[qtr])

            nc.sync.dma_start(cst_sb[:], cst_d[:])
            load_qtr(xq, qt_d, 0, nc.sync)
            load_qtr(xk, kt_d, 0, nc.sync)

            def defer_load(x_sb, src_d, qtr, gate):
                """Hold quarter `qtr`'s load until `gate` (an SBUF cell) is
                written: a 1-element Pool copy into the quarter's region
                creates a WAW dep on the DMA, serializing DMA traffic so
                early-needed transfers get the full bandwidth."""
                c0 = qtr * 512
                nc.gpsimd.tensor_copy(x_sb[0:1, 0, c0:c0 + 1], gate)
                load_qtr(x_sb, src_d, qtr, nc.sync)

            # ---- PE warm-up: junk matmuls while the first DMAs fly --------
            nc.vector.memset(wu[:], 0.0)
            wu_ps = ps.tile([128, 512], F32, tag="sc", name="wu_ps", bufs=4)
            for i in range(7):
                nc.tensor.matmul(wu_ps[:], lhsT=wu[:, 0:128], rhs=wu[:],
                                 start=(i == 0), stop=(i == 6))
            nc.vector.tensor_copy(wu2[:], wu_ps[:, 0:1])

            # ones columns for v' (Pool, early, no deps)
            for p in range(4):
                nc.gpsimd.memset(vp4[p][:, :, 64:66], 1.0)

            # ---- emission units -------------------------------------------
            def query_proj(p):
                sl = slice(p * 512, (p + 1) * 512)
                pq = ps.tile([DK, 512], F32, tag="sc", name=f"pq_{p}", bufs=4)
                for cc in range(4):
                    nc.tensor.matmul(pq[:],
                                     lhsT=wq_sb[:, cc * DK:(cc + 1) * DK],
                                     rhs=xq[:, cc, sl],
                                     start=(cc == 0), stop=(cc == 3))
                nc.vector.tensor_scalar_add(qT[:, sl], pq[:], bq_sb[:, 0:1])

            def kv_proj(p):
                sl = slice(p * 512, (p + 1) * 512)
                pkv = ps.tile([128, 512], F32, tag="sc", name=f"pkv_{p}", bufs=4)
                for cc in range(4):
                    nc.tensor.matmul(pkv[:],
                                     lhsT=wkv_sb[:, cc * 128:(cc + 1) * 128],
                                     rhs=xk[:, cc, sl],
                                     start=(cc == 0), stop=(cc == 3))
                nc.vector.tensor_scalar_add(kvT[:, sl], pkv[:], bkv_sb[:, 0:1])

            def vprime_units(p):
                """v' for sweep p as two filler units (2 transposes each)
                plus the eviction."""
                pv4 = ps.tile([128, 4, DK], BF, tag="pl", name=f"pv4_{p}", bufs=1)

                def half(h):
                    def emit():
                        for t in (2 * h, 2 * h + 1):
                            jb = p * 4 + t
                            nc.tensor.transpose(
                                pv4[:, t, :],
                                kvT[64:128, jb * 128:(jb + 1) * 128],
                                id_sb[64:128, 64:128])
                        if h == 1:
                            nc.vector.tensor_copy(vp4[p][:, :, 0:DK], pv4[:])
                    return emit
                return [half(0), half(1)]

            def sc_unit(J, p):
                pt = pts[J]
                i_start = max(512 * p, 128 * J) if masked else 512 * p
                w = 512 * p + 512 - i_start
                x0 = i_start - (128 * J if masked else 0)
                psc = ps.tile([128, 512], F32, tag="sc", name=f"sc_{J}_{p}",
                              bufs=4)
                nc.tensor.matmul(psc[:, 0:w],
                                 lhsT=kvT[0:DK, J * 128:(J + 1) * 128],
                                 rhs=qT[:, i_start:i_start + w],
                                 start=True, stop=True,
                                 skip_group_check=True)
                nc.scalar.activation(pt[:, x0:x0 + w], psc[:, 0:w],
                                     Exp, scale=0.125)
                if masked and J // 4 == p:
                    nc.gpsimd.tensor_mul(pt[:, 0:128], pt[:, 0:128],
                                         ltmask_sb[:])

            def hd_unit(J, p, hacc, Jmax):
                b_lo = max(4 * p, J) if masked else 4 * p
                wdt = (4 * p + 4 - b_lo) * 128
                c0 = (b_lo % 4) * 128
                x = (128 * (b_lo - J) if masked else 512 * p)
                nc.tensor.matmul(hacc[:, c0:c0 + wdt],
                                 lhsT=vp4[J // 4][:, J % 4, 0:DK + 1],
                                 rhs=pts[J][:, x:x + wdt],
                                 start=(J == 0), stop=(J == Jmax),
                                 skip_group_check=True)

            def sweep(p, pending, fillers):
                """scores for sweep p with: previous sweep's pending units
                (tail heads + ht4 evict) first, then filler units, one per
                score slot.  Own heads start once pending has drained and
                trail by >=3 slots; leftover heads spill into the NEXT
                sweep's pending (returned)."""
                Jmax = 4 * p + 3 if masked else NB - 1
                hacc = ps.tile([DK + 1, 512], F32, tag="ha", name=f"ha_{p}",
                               bufs=1)
                pend = list(pending)
                fill = list(fillers)
                lag = max(3, len(pend) + 1)
                emitted_h = 0
                for J in range(0, Jmax + 1):
                    sc_unit(J, p)
                    if pend:
                        pend.pop(0)()
                    elif fill:
                        fill.pop(0)()
                    if J >= lag:
                        hd_unit(emitted_h, p, hacc, Jmax)
                        emitted_h += 1
                for f in pend + fill:
                    f()
                ht4 = hts.tile([DK + 1, 512], BF, tag="ht", name=f"ht4_{p}")
                next_pending = []
                h0 = emitted_h

                def mk_hd(J):
                    def emit():
                        hd_unit(J, p, hacc, Jmax)
                    return emit

                def mk_ht4():
                    nc.vector.tensor_copy(ht4[:], hacc[:])

                for J in range(h0, Jmax + 1):
                    next_pending.append(mk_hd(J))
                next_pending.append(mk_ht4)
                return ht4, next_pending

            def finalize_units(p, ht4):
                """finalize sweep p as filler units: l-extract, 4 out blocks,
                store."""
                ot4 = osb.tile([128, 4, D], BF, tag="ot", name=f"ot4_{p}")
                pl4 = ps.tile([128, 4, 2], BF, tag="pl", name=f"pl4_{p}", bufs=1)
                r4 = hts.tile([128, 4], F32, tag="r", name=f"r4_{p}")

                def lt():
                    for t in range(4):
                        nc.tensor.transpose(pl4[:, t, 0:1],
                                            ht4[DK:DK + 1, t * 128:(t + 1) * 128],
                                            id_sb[64:65, 64:65])
                    nc.vector.reciprocal(r4[:], pl4[:, :, 0])

                def out_block(b):
                    def emit():
                        c0 = (b % 4) * 128
                        po = ps.tile([128, 512], F32, tag="po", name=f"po_{b}",
                                     bufs=2)
                        nc.tensor.matmul(po[:], lhsT=ht4[:, c0:c0 + 128],
                                         rhs=frhs_sb[:], start=True, stop=True)
                        nc.vector.tensor_scalar_mul(ot4[:, b % 4, :], po[:],
                                                    r4[:, b % 4:b % 4 + 1])
                    return emit

                def store():
                    nc.sync.dma_start(
                        out_d[p * 512:(p + 1) * 512, :].rearrange(
                            "(c p) i -> p c i", p=128),
                        ot4[:])

                return [lt] + [out_block(b) for b in range(4 * p, 4 * p + 4)] \
                    + [store]

            # ---- program --------------------------------------------------
            query_proj(0)
            # chain the remaining loads pairwise: each pair gated on earlier
            # landed data so early-needed transfers get full bandwidth
            defer_load(xq, qt_d, 1, qT[0:1, 0:1])
            defer_load(xk, kt_d, 1, qT[0:1, 1:2])
            defer_load(xq, qt_d, 2, xk[0:1, 1, 512:513])
            defer_load(xk, kt_d, 2, xk[0:1, 2, 512:513])
            defer_load(xq, qt_d, 3, xk[0:1, 1, 1024:1025])
            defer_load(xk, kt_d, 3, xk[0:1, 2, 1024:1025])
            kv_proj(0)
            for f in vprime_units(0):
                f()

            def proj_unit(p):
                def emit():
                    query_proj(p)
                return emit

            def kv_unit(p):
                def emit():
                    kv_proj(p)
                    for f in vprime_units(p):
                        f()
                return emit

            ht0, pend = sweep(0, [], [proj_unit(1), kv_unit(1)])
            ht1, pend = sweep(1, pend, [proj_unit(2), kv_unit(2)]
                              + finalize_units(0, ht0))
            ht2, pend = sweep(2, pend, [proj_unit(3), kv_unit(3)]
                              + finalize_units(1, ht1))
            ht3, pend = sweep(3, pend, finalize_units(2, ht2))
            for f in pend:
                f()
            for f in finalize_units(3, ht3):
                f()
            nc.gpsimd.dma_start(warm_d[:], wu2[:])

    _split_sync_waits(nc)
    return nc


_NC_CACHE = {}


def _get_nc(masked: bool):
    if masked not in _NC_CACHE:
        _NC_CACHE[masked] = _build_nc(masked)
    return _NC_CACHE[masked]


def _prep_consts(Wq, bq, Wk, bk, Wv, bv, Wo, bo):
    Wq = np.asarray(Wq, dtype=np.float64)
    Wk = np.asarray(Wk, dtype=np.float64)
    Wv = np.asarray(Wv, dtype=np.float64)
    Wo = np.asarray(Wo, dtype=np.float64)
    bq_h = np.asarray(bq, dtype=np.float32).reshape(DK, 1)
    bk_h = np.asarray(bk, dtype=np.float32).reshape(DK, 1)
    bv_h = np.asarray(bv, dtype=np.float32).reshape(DK, 1)
    bo_h = np.asarray(bo, dtype=np.float64)
    wo_eff = Wo.reshape(H, DK, D).sum(axis=0)
    frhs_h = np.concatenate([wo_eff, bo_h[None, :]], axis=0).astype(ml_dtypes.bfloat16)
    jj, ii = np.meshgrid(np.arange(128), np.arange(128), indexing="ij")
    ltmask_h = (jj <= ii).astype(ml_dtypes.bfloat16)
    wq_bf = Wq.astype(ml_dtypes.bfloat16)                       # [512, 64]
    wkv_bf = np.concatenate([Wk, Wv], axis=1).astype(ml_dtypes.bfloat16)

    cst = np.zeros((128, _C_TOT), dtype=ml_dtypes.bfloat16)
    cst[0:DK + 1, _C_FRHS:_C_FRHS + 512] = frhs_h
    cst[:, _C_MASK:_C_MASK + 128] = ltmask_h
    cst[:, _C_ID:_C_ID + 128] = np.eye(128, dtype=ml_dtypes.bfloat16)
    for cc in range(4):
        cst[:, _C_WQ + cc * DK:_C_WQ + (cc + 1) * DK] = \
            wq_bf[cc * 128:(cc + 1) * 128]
        cst[:, _C_WKV + cc * 128:_C_WKV + (cc + 1) * 128] = \
            wkv_bf[cc * 128:(cc + 1) * 128]
    cst[0:DK, _C_BIAS:_C_BIAS + 2] = np.ascontiguousarray(bq_h).view(
        ml_dtypes.bfloat16)
    bkv_f = np.ascontiguousarray(np.concatenate([bk_h, bv_h], axis=0))
    cst[:, _C_BIAS + 2:_C_BIAS + 4] = bkv_f.view(ml_dtypes.bfloat16)
    return {"cst": cst}


def _to_input(xt):
    """[512, S] f32 (X^T) -> bf16 quarter-major [4, 128, 4, 512]."""
    x = xt.reshape(4, 128, 4, 512)               # [chunk, p, qtr, col]
    return np.ascontiguousarray(
        x.transpose(2, 1, 0, 3)                  # [qtr, p, chunk, col]
    ).astype(ml_dtypes.bfloat16)


# ---------------------------------------------------------------------------
def kernel(query, key, value, Wq, bq, Wk, bk, Wv, bv, Wo, bo, training):
    query = np.asarray(query, dtype=np.float32)
    key = np.asarray(key, dtype=np.float32)
    masked = bool(np.asarray(training).item())
    B = query.shape[0]

    consts = _prep_consts(Wq, bq, Wk, bk, Wv, bv, Wo, bo)
    in_maps = [
        dict(consts,
             qt=_to_input(np.ascontiguousarray(query[i].T)),
             kt=_to_input(np.ascontiguousarray(key[i].T)))
        for i in range(B)
    ]

    nc = _get_nc(masked)
    res = run_bass_kernel_spmd(nc, in_maps, core_ids=list(range(B)))
    return np.stack([np.asarray(res.results[i]["out"]).astype(np.float32)
                     for i in range(B)])
